# revision 45
# baseline (speedup 1.0000x reference)
"""TRN2 Bass kernel for nn_AttentionModuleV1 (gnn_message_passing).

Math note: the reference broadcasts features to a neighbor axis L=16 where
every slice is identical, so softmax over L is exactly uniform (1/16) and
the module collapses to (per row n of the N=16384 point axis):

    pos = relu(features  @ Wk.T)
    h   = relu(bn1(features2 @ Wv1.T))
    val = relu(bn2(h @ Wv2.T))
    vc  = sigmoid(pos @ Wv_coef.T)
    out = val + pos * vc

(xyz, Wa, Wq_coef, Wk_coef, Wqk_coef do not affect the output: they only
feed the softmax logits, which are constant along L.)

Sharding: pure data parallel over N across 8 cores (2048 rows each).
On-chip layout is feature-major ([C, n] with C on partitions, split into
two 128-partition chunks); inputs are transposed on the host, weights are
passed pre-transposed, and BN is folded into per-channel scale/bias.
"""
import sys

sys.path.insert(0, "/opt/trn_rl_repo")

import numpy as np
from concourse import bacc, mybir
import concourse.tile as tile
from concourse.tile import add_dep_helper
from concourse.bass_utils import run_bass_kernel_spmd

F32 = mybir.dt.float32
F32R = mybir.dt.float32r
BF16 = mybir.dt.bfloat16
AF = mybir.ActivationFunctionType

N_TOTAL = 16384
C = 256        # input feature channels
OUT = 256      # output channels
NCORES = 8
NSH = N_TOTAL // NCORES   # 2048 rows per core
P = 128
KC = C // P    # contraction chunks
OC = OUT // P  # output-channel chunks
NTILE = 1024   # n-columns per pipeline tile
MMN = 512      # moving free dim per matmul (PSUM bank = 512 fp32)
NJ = NTILE // MMN
NT = NSH // NTILE
BN_EPS = 1e-5

# matmul operand mode: "f32" (4 cyc/row, exact), "f32r" (1 cyc/row,
# ~1.5e-4 rel err measured on HW), "bf16" (1 cyc/row, ~2.4e-3 rel err)
MODE = "f32r"

_cache = {}


def _build(mode):
    mdt = {"f32": F32, "f32r": F32R, "bf16": BF16}[mode]
    # Inputs consumed by matmuls are declared with the matmul dtype so the
    # load is a plain (fast, HWDGE) DMA and the operand-producer dtype
    # satisfies the FP32r rounding rule. For f32r the bits are plain fp32
    # (np dtype float32 either way).
    xdt = mdt if mode != "bf16" else F32
    nc = bacc.Bacc(None, target_bir_lowering=False, debug=True)

    x1_d = nc.declare_dram_parameter("x1t", [C, NSH], xdt, isOutput=False)
    x2_d = nc.declare_dram_parameter("x2t", [C, NSH], xdt, isOutput=False)
    wk_d = nc.declare_dram_parameter("wkT", [C, OUT], xdt, isOutput=False)
    wv1_d = nc.declare_dram_parameter("wv1T", [C, OUT], xdt, isOutput=False)
    wv2_d = nc.declare_dram_parameter("wv2T", [OUT, OUT], xdt, isOutput=False)
    wvc_d = nc.declare_dram_parameter("wvcT", [OUT, OUT], xdt, isOutput=False)
    sb_d = nc.declare_dram_parameter("sb", [OUT, 4], F32, isOutput=False)
    out_d = nc.declare_dram_parameter("outT", [OUT, NSH], F32, isOutput=True)

    with tile.TileContext(nc) as tc:
        with (
            tc.tile_pool(name="wpool", bufs=1) as wpool,
            tc.tile_pool(name="inpool", bufs=1) as inpool,
            tc.tile_pool(name="midpool", bufs=2) as midpool,
            tc.tile_pool(name="outpool", bufs=2) as outpool,
            tc.tile_pool(name="psum", bufs=1, space="PSUM") as psum,
        ):
            # -- PE warmup burst: keep the PE busy from the preamble's end
            # so the HAM clock gate opens sooner once real matmuls start.
            scratch = wpool.tile([P, MMN], BF16, tag="scratch")
            nc.gpsimd.memset(scratch, 0.0)
            wacc = psum.tile([P, MMN], F32, tag="acc", bufs=4)
            for _ in range(14):
                nc.tensor.matmul(wacc, scratch[:, :P], scratch,
                                 start=True, stop=True)
            # dummy sigmoid: hoist the (relu+sigmoid) ACT table load into
            # the DMA ramp instead of the first real activation
            dumm = wpool.tile([P, 1], F32, tag="dumm")
            nc.scalar.activation(dumm, scratch.bitcast(F32)[:, 0:1],
                                 AF.Sigmoid)

            def load_x(d, it, name):
                nsl = slice(it * NTILE, (it + 1) * NTILE)
                t = inpool.tile([P, KC, NTILE], mdt, tag=name)
                eng = nc.gpsimd if mode == "bf16" else nc.sync
                bi = eng.dma_start(
                    out=t,
                    in_=d.ap()[:, nsl].rearrange("(kc p) n -> p kc n", p=P))
                return t, bi

            x1, x1t0_bi = load_x(x1_d, 0, "x1")
            x2, x2t0_bi = load_x(x2_d, 0, "x2")

            def load_w(d, name, after=None):
                t = wpool.tile([P, KC, OUT], mdt, tag=name)
                bi = nc.gpsimd.dma_start(
                    out=t, in_=d.ap().rearrange("(kc p) o -> p kc o", p=P))
                if after is not None:
                    add_dep_helper(bi.ins, after.ins, sync=True,
                                   reason="late weights yield ramp bandwidth")
                return t

            # sbt is tiny and feeds the h/val bias drains early - load first
            sbt = wpool.tile([P, OC, 4], F32, tag="sbt")
            nc.gpsimd.dma_start(
                out=sbt, in_=sb_d.ap().rearrange("(oc p) c -> p oc c", p=P))
            wk = load_w(wk_d, "wk")
            # wv1/wvc/wv2 are not needed until well into tile 0; keep them
            # out of the critical first-load window
            wv1 = load_w(wv1_d, "wv1", after=x1t0_bi)
            wvc = load_w(wvc_d, "wvc", after=x1t0_bi)
            wv2 = load_w(wv2_d, "wv2", after=x1t0_bi)

            def mm_group(w, rhs_tile, oc):
                # one PSUM region [P, NTILE] (NJ banks), KC*NJ matmuls
                acc = psum.tile([P, NTILE], F32, tag="acc", bufs=4)
                size = NTILE
                first = None
                for kc in range(KC):
                    for j in range(size // MMN):
                        bi = nc.tensor.matmul(
                            acc[:, j * MMN:(j + 1) * MMN],
                            w[:, kc, oc * P:(oc + 1) * P],
                            rhs_tile[:, kc, j * MMN:(j + 1) * MMN],
                            start=(kc == 0), stop=(kc == KC - 1))
                        first = first or bi
                return acc, first

            def as_f32(ap):
                return ap.bitcast(F32) if mode == "f32r" else ap

            for it in range(NT):
                nsl = slice(it * NTILE, (it + 1) * NTILE)
                if it > 0:
                    x1, _ = load_x(x1_d, it, "x1")
                    x2, _ = load_x(x2_d, it, "x2")

                pos = midpool.tile([P, OC, NTILE], mdt, tag="pos")
                h = midpool.tile([P, OC, NTILE], mdt, tag="h")
                vc = midpool.tile([P, OC, NTILE], F32, tag="vc")
                val = midpool.tile([P, OC, NTILE], F32, tag="val")
                prod = outpool.tile([P, OC, NTILE], F32, tag="prod")
                outt = outpool.tile([P, OC, NTILE], F32, tag="outt")

                # pos = relu(Wk @ x1)           (DVE: max(acc, 0))
                for oc in range(OC):
                    acc, fmm = mm_group(wk, x1, oc)
                    if it == 0 and oc == 0:
                        # x2t0 is not needed until the h layer; keep it out
                        # of the critical x1t0+wk ramp window
                        add_dep_helper(x2t0_bi.ins, fmm.ins, sync=True,
                                       reason="defer x2t0 past ramp")
                    nc.vector.tensor_scalar_max(pos[:, oc, :], acc, 0.0)
                # h = relu((s1*Wv1) @ x2 + b1)  (s1 folded on host; drains
                # split between ACT and DVE to balance engine load)
                for oc in range(OC):
                    acc, _ = mm_group(wv1, x2, oc)
                    if oc == 0:
                        nc.scalar.activation(h[:, oc, :], acc, AF.Relu,
                                             bias=sbt[:, oc, 0:1])
                    else:
                        nc.vector.tensor_scalar(h[:, oc, :], acc,
                                                sbt[:, oc, 0:1], 0.0,
                                                mybir.AluOpType.add,
                                                mybir.AluOpType.max)
                # vc = sigmoid(Wvc @ pos)       (ACT)
                for oc in range(OC):
                    acc, _ = mm_group(wvc, pos, oc)
                    nc.scalar.activation(vc[:, oc, :], acc, AF.Sigmoid)
                # val = relu((s2*Wv2) @ h + b2) (s2 folded on host; ACT)
                for oc in range(OC):
                    acc, _ = mm_group(wv2, h, oc)
                    nc.scalar.activation(val[:, oc, :], acc, AF.Relu,
                                         bias=sbt[:, oc, 2:3])
                # out = val + pos * vc  (mul DVE; adds split GpSimd/DVE on
                # the last tile so the tail runs in parallel)
                for oc in range(OC):
                    nc.vector.tensor_mul(prod[:, oc, :],
                                         as_f32(pos[:, oc, :]), vc[:, oc, :])
                last = it == NT - 1
                if not last:
                    for oc in range(OC):
                        nc.gpsimd.tensor_add(outt[:, oc, :], prod[:, oc, :],
                                             val[:, oc, :])
                        nc.sync.dma_start(
                            out=out_d.ap()[oc * P:(oc + 1) * P, nsl],
                            in_=outt[:, oc, :])
                else:
                    # tail: split adds into halves across GpSimd+DVE so the
                    # final add/store chain runs in parallel
                    for oc in range(OC):
                        for jh in range(2):
                            hs = slice(jh * (NTILE // 2), (jh + 1) * (NTILE // 2))
                            eng = nc.gpsimd if (oc == 0 and jh == 0) else nc.vector
                            eng.tensor_add(outt[:, oc, hs], prod[:, oc, hs],
                                           val[:, oc, hs])
                            dsl = slice(it * NTILE + jh * (NTILE // 2),
                                        it * NTILE + (jh + 1) * (NTILE // 2))
                            nc.sync.dma_start(
                                out=out_d.ap()[oc * P:(oc + 1) * P, dsl],
                                in_=outt[:, oc, hs])
    nc.finalize()
    return nc


def _prep(inputs):
    f = np.ascontiguousarray(np.asarray(inputs["features"], np.float32).T)
    f2 = np.ascontiguousarray(np.asarray(inputs["features2"], np.float32).T)
    wkT = np.ascontiguousarray(np.asarray(inputs["Wk"], np.float32).T)
    wvcT = np.ascontiguousarray(np.asarray(inputs["Wv_coef"], np.float32).T)

    eps = np.float32(BN_EPS)
    s1 = np.asarray(inputs["bn1_g"], np.float32) / np.sqrt(
        np.asarray(inputs["bn1_v"], np.float32) + eps)
    b1 = np.asarray(inputs["bn1_b"], np.float32) - np.asarray(
        inputs["bn1_m"], np.float32) * s1
    s2 = np.asarray(inputs["bn2_g"], np.float32) / np.sqrt(
        np.asarray(inputs["bn2_v"], np.float32) + eps)
    b2 = np.asarray(inputs["bn2_b"], np.float32) - np.asarray(
        inputs["bn2_m"], np.float32) * s2
    # bn scales fold into Wv1/Wv2 (per-output-channel row scales); biases
    # are applied on-chip.  sb columns: [b1, unused, b2, unused]
    wv1T = np.ascontiguousarray(
        (np.asarray(inputs["Wv1"], np.float32) * s1[:, None]).T)
    wv2T = np.ascontiguousarray(
        (np.asarray(inputs["Wv2"], np.float32) * s2[:, None]).T)
    sb = np.ascontiguousarray(
        np.stack([b1, s2, b2, np.zeros_like(b1)], axis=1).astype(np.float32))

    in_maps = []
    for i in range(NCORES):
        nsl = slice(i * NSH, (i + 1) * NSH)
        in_maps.append({
            "x1t": np.ascontiguousarray(f[:, nsl]),
            "x2t": np.ascontiguousarray(f2[:, nsl]),
            "wkT": wkT, "wv1T": wv1T, "wv2T": wv2T, "wvcT": wvcT,
            "sb": sb,
        })
    return in_maps


def _run(inputs, trace=False, trace_cores=None, tmpdir=None):
    if MODE not in _cache:
        _cache[MODE] = _build(MODE)
    nc = _cache[MODE]
    in_maps = _prep(inputs)
    kw = {}
    if trace:
        kw = dict(trace=True, trace_cores=trace_cores or [0], tmpdir=tmpdir)
    res = run_bass_kernel_spmd(nc, in_maps, core_ids=list(range(NCORES)), **kw)
    out = np.empty((N_TOTAL, OUT), np.float32)
    for i in range(NCORES):
        out[i * NSH:(i + 1) * NSH, :] = res.results[i]["outT"].T
    return out, res


def kernel(**inputs):
    out, _ = _run(inputs, trace=False)
    return out


# revision 47
# speedup vs baseline: 1.0443x; 1.0443x over previous
"""TRN2 Bass kernel for nn_AttentionModuleV1 (gnn_message_passing).

Math note: the reference broadcasts features to a neighbor axis L=16 where
every slice is identical, so softmax over L is exactly uniform (1/16) and
the module collapses to (per row n of the N=16384 point axis):

    pos = relu(features  @ Wk.T)
    h   = relu(bn1(features2 @ Wv1.T))
    val = relu(bn2(h @ Wv2.T))
    vc  = sigmoid(pos @ Wv_coef.T)
    out = val + pos * vc

(xyz, Wa, Wq_coef, Wk_coef, Wqk_coef do not affect the output: they only
feed the softmax logits, which are constant along L.)

Sharding: pure data parallel over N across 8 cores (2048 rows each).
On-chip layout is feature-major ([C, n] with C on partitions, split into
two 128-partition chunks); inputs are transposed on the host, weights are
passed pre-transposed, and BN is folded into per-channel scale/bias.
"""
import sys

sys.path.insert(0, "/opt/trn_rl_repo")

import numpy as np
from concourse import bacc, mybir
import concourse.tile as tile
from concourse.tile import add_dep_helper
from concourse.bass_utils import run_bass_kernel_spmd

F32 = mybir.dt.float32
F32R = mybir.dt.float32r
BF16 = mybir.dt.bfloat16
AF = mybir.ActivationFunctionType

N_TOTAL = 16384
C = 256        # input feature channels
OUT = 256      # output channels
NCORES = 8
NSH = N_TOTAL // NCORES   # 2048 rows per core
P = 128
KC = C // P    # contraction chunks
OC = OUT // P  # output-channel chunks
NTILE = 1024   # n-columns per pipeline tile
MMN = 512      # moving free dim per matmul (PSUM bank = 512 fp32)
NJ = NTILE // MMN
NT = NSH // NTILE
BN_EPS = 1e-5

# matmul operand mode: "f32" (4 cyc/row, exact), "f32r" (1 cyc/row,
# ~1.5e-4 rel err measured on HW), "bf16" (1 cyc/row, ~2.4e-3 rel err)
MODE = "f32r"

_cache = {}


def _build(mode):
    mdt = {"f32": F32, "f32r": F32R, "bf16": BF16}[mode]
    # Inputs consumed by matmuls are declared with the matmul dtype so the
    # load is a plain (fast, HWDGE) DMA and the operand-producer dtype
    # satisfies the FP32r rounding rule. For f32r the bits are plain fp32
    # (np dtype float32 either way).
    xdt = mdt if mode != "bf16" else F32
    nc = bacc.Bacc(None, target_bir_lowering=False, debug=True)

    x1_d = nc.declare_dram_parameter("x1t", [C, NSH], xdt, isOutput=False)
    x2_d = nc.declare_dram_parameter("x2t", [C, NSH], xdt, isOutput=False)
    wk_d = nc.declare_dram_parameter("wkT", [C, OUT], xdt, isOutput=False)
    wv1_d = nc.declare_dram_parameter("wv1T", [C, OUT], xdt, isOutput=False)
    wv2_d = nc.declare_dram_parameter("wv2T", [OUT, OUT], xdt, isOutput=False)
    wvc_d = nc.declare_dram_parameter("wvcT", [OUT, OUT], xdt, isOutput=False)
    sb_d = nc.declare_dram_parameter("sb", [OUT, 4], F32, isOutput=False)
    out_d = nc.declare_dram_parameter("outT", [OUT, NSH], F32, isOutput=True)

    with tile.TileContext(nc) as tc:
        with (
            tc.tile_pool(name="wpool", bufs=1) as wpool,
            tc.tile_pool(name="inpool", bufs=1) as inpool,
            tc.tile_pool(name="midpool", bufs=3) as midpool,
            tc.tile_pool(name="outpool", bufs=2) as outpool,
            tc.tile_pool(name="psum", bufs=1, space="PSUM") as psum,
        ):
            # -- PE warmup burst: keep the PE busy from the preamble's end
            # so the HAM clock gate opens sooner once real matmuls start.
            scratch = wpool.tile([P, MMN], BF16, tag="scratch")
            nc.gpsimd.memset(scratch, 0.0)
            wacc = psum.tile([P, MMN], F32, tag="acc", bufs=4)
            for _ in range(8):
                nc.tensor.matmul(wacc, scratch[:, :P], scratch,
                                 start=True, stop=True)
            # dummy sigmoid: hoist the (relu+sigmoid) ACT table load into
            # the DMA ramp instead of the first real activation
            dumm = wpool.tile([P, 1], F32, tag="dumm")
            nc.scalar.activation(dumm, scratch.bitcast(F32)[:, 0:1],
                                 AF.Sigmoid)

            def load_x(d, it, name):
                nsl = slice(it * NTILE, (it + 1) * NTILE)
                t = inpool.tile([P, KC, NTILE], mdt, tag=name)
                eng = nc.gpsimd if mode == "bf16" else nc.sync
                bi = eng.dma_start(
                    out=t,
                    in_=d.ap()[:, nsl].rearrange("(kc p) n -> p kc n", p=P))
                return t, bi

            x1, _ = load_x(x1_d, 0, "x1")
            x2, _ = load_x(x2_d, 0, "x2")

            def load_w(d, name):
                t = wpool.tile([P, KC, OUT], mdt, tag=name)
                nc.gpsimd.dma_start(
                    out=t, in_=d.ap().rearrange("(kc p) o -> p kc o", p=P))
                return t

            # sbt is tiny and feeds the h/val bias drains early - load first
            sbt = wpool.tile([P, OC, 4], F32, tag="sbt")
            nc.gpsimd.dma_start(
                out=sbt, in_=sb_d.ap().rearrange("(oc p) c -> p oc c", p=P))
            wk = load_w(wk_d, "wk")
            wv1 = load_w(wv1_d, "wv1")
            wvc = load_w(wvc_d, "wvc")
            wv2 = load_w(wv2_d, "wv2")

            def mm_group(w, rhs_tile, oc):
                # one PSUM region [P, NTILE] (NJ banks), KC*NJ matmuls
                acc = psum.tile([P, NTILE], F32, tag="acc", bufs=4)
                size = NTILE
                first = None
                for kc in range(KC):
                    for j in range(size // MMN):
                        bi = nc.tensor.matmul(
                            acc[:, j * MMN:(j + 1) * MMN],
                            w[:, kc, oc * P:(oc + 1) * P],
                            rhs_tile[:, kc, j * MMN:(j + 1) * MMN],
                            start=(kc == 0), stop=(kc == KC - 1))
                        first = first or bi
                return acc, first

            def as_f32(ap):
                return ap.bitcast(F32) if mode == "f32r" else ap

            for it in range(NT):
                nsl = slice(it * NTILE, (it + 1) * NTILE)
                if it > 0:
                    x1, _ = load_x(x1_d, it, "x1")
                    x2, _ = load_x(x2_d, it, "x2")

                pos = midpool.tile([P, OC, NTILE], mdt, tag="pos")
                h = midpool.tile([P, OC, NTILE], mdt, tag="h")
                vc = midpool.tile([P, OC, NTILE], F32, tag="vc")
                val = midpool.tile([P, OC, NTILE], F32, tag="val")
                prod = outpool.tile([P, OC, NTILE], F32, tag="prod")
                outt = outpool.tile([P, OC, NTILE], F32, tag="outt")

                # pos = relu(Wk @ x1)           (DVE: max(acc, 0))
                for oc in range(OC):
                    acc, _ = mm_group(wk, x1, oc)
                    nc.vector.tensor_scalar_max(pos[:, oc, :], acc, 0.0)
                # h = relu((s1*Wv1) @ x2 + b1)  (s1 folded on host; drains
                # split between ACT and DVE to balance engine load)
                for oc in range(OC):
                    acc, _ = mm_group(wv1, x2, oc)
                    if oc == 0:
                        nc.scalar.activation(h[:, oc, :], acc, AF.Relu,
                                             bias=sbt[:, oc, 0:1])
                    else:
                        nc.vector.tensor_scalar(h[:, oc, :], acc,
                                                sbt[:, oc, 0:1], 0.0,
                                                mybir.AluOpType.add,
                                                mybir.AluOpType.max)
                # vc = sigmoid(Wvc @ pos)       (ACT)
                for oc in range(OC):
                    acc, _ = mm_group(wvc, pos, oc)
                    nc.scalar.activation(vc[:, oc, :], acc, AF.Sigmoid)
                # val = relu((s2*Wv2) @ h + b2) (s2 folded on host; ACT)
                for oc in range(OC):
                    acc, _ = mm_group(wv2, h, oc)
                    nc.scalar.activation(val[:, oc, :], acc, AF.Relu,
                                         bias=sbt[:, oc, 2:3])
                # out = val + pos * vc  (mul DVE; adds split GpSimd/DVE on
                # the last tile so the tail runs in parallel)
                for oc in range(OC):
                    nc.vector.tensor_mul(prod[:, oc, :],
                                         as_f32(pos[:, oc, :]), vc[:, oc, :])
                last = it == NT - 1
                for oc in range(OC):
                    eng = nc.vector if (last and oc == 1) else nc.gpsimd
                    eng.tensor_add(outt[:, oc, :], prod[:, oc, :],
                                   val[:, oc, :])
                    # stores ride the second HWDGE ring (ACT-issued) so
                    # mid-kernel x-loads never queue behind them
                    nc.scalar.dma_start(
                        out=out_d.ap()[oc * P:(oc + 1) * P, nsl],
                        in_=outt[:, oc, :])
    nc.finalize()
    return nc


def _prep(inputs):
    f = np.ascontiguousarray(np.asarray(inputs["features"], np.float32).T)
    f2 = np.ascontiguousarray(np.asarray(inputs["features2"], np.float32).T)
    wkT = np.ascontiguousarray(np.asarray(inputs["Wk"], np.float32).T)
    wvcT = np.ascontiguousarray(np.asarray(inputs["Wv_coef"], np.float32).T)

    eps = np.float32(BN_EPS)
    s1 = np.asarray(inputs["bn1_g"], np.float32) / np.sqrt(
        np.asarray(inputs["bn1_v"], np.float32) + eps)
    b1 = np.asarray(inputs["bn1_b"], np.float32) - np.asarray(
        inputs["bn1_m"], np.float32) * s1
    s2 = np.asarray(inputs["bn2_g"], np.float32) / np.sqrt(
        np.asarray(inputs["bn2_v"], np.float32) + eps)
    b2 = np.asarray(inputs["bn2_b"], np.float32) - np.asarray(
        inputs["bn2_m"], np.float32) * s2
    # bn scales fold into Wv1/Wv2 (per-output-channel row scales); biases
    # are applied on-chip.  sb columns: [b1, unused, b2, unused]
    wv1T = np.ascontiguousarray(
        (np.asarray(inputs["Wv1"], np.float32) * s1[:, None]).T)
    wv2T = np.ascontiguousarray(
        (np.asarray(inputs["Wv2"], np.float32) * s2[:, None]).T)
    sb = np.ascontiguousarray(
        np.stack([b1, s2, b2, np.zeros_like(b1)], axis=1).astype(np.float32))

    in_maps = []
    for i in range(NCORES):
        nsl = slice(i * NSH, (i + 1) * NSH)
        in_maps.append({
            "x1t": np.ascontiguousarray(f[:, nsl]),
            "x2t": np.ascontiguousarray(f2[:, nsl]),
            "wkT": wkT, "wv1T": wv1T, "wv2T": wv2T, "wvcT": wvcT,
            "sb": sb,
        })
    return in_maps


def _run(inputs, trace=False, trace_cores=None, tmpdir=None):
    if MODE not in _cache:
        _cache[MODE] = _build(MODE)
    nc = _cache[MODE]
    in_maps = _prep(inputs)
    kw = {}
    if trace:
        kw = dict(trace=True, trace_cores=trace_cores or [0], tmpdir=tmpdir)
    res = run_bass_kernel_spmd(nc, in_maps, core_ids=list(range(NCORES)), **kw)
    out = np.empty((N_TOTAL, OUT), np.float32)
    for i in range(NCORES):
        out[i * NSH:(i + 1) * NSH, :] = res.results[i]["outT"].T
    return out, res


def kernel(**inputs):
    out, _ = _run(inputs, trace=False)
    return out


# revision 48
# speedup vs baseline: 1.0765x; 1.0308x over previous
"""TRN2 Bass kernel for nn_AttentionModuleV1 (gnn_message_passing).

Math note: the reference broadcasts features to a neighbor axis L=16 where
every slice is identical, so softmax over L is exactly uniform (1/16) and
the module collapses to (per row n of the N=16384 point axis):

    pos = relu(features  @ Wk.T)
    h   = relu(bn1(features2 @ Wv1.T))
    val = relu(bn2(h @ Wv2.T))
    vc  = sigmoid(pos @ Wv_coef.T)
    out = val + pos * vc

(xyz, Wa, Wq_coef, Wk_coef, Wqk_coef do not affect the output: they only
feed the softmax logits, which are constant along L.)

Sharding: pure data parallel over N across 8 cores (2048 rows each).
On-chip layout is feature-major ([C, n] with C on partitions, split into
two 128-partition chunks); inputs are transposed on the host, weights are
passed pre-transposed, and BN is folded into per-channel scale/bias.
"""
import sys

sys.path.insert(0, "/opt/trn_rl_repo")

import numpy as np
from concourse import bacc, mybir
import concourse.tile as tile
from concourse.tile import add_dep_helper
from concourse.bass_utils import run_bass_kernel_spmd

F32 = mybir.dt.float32
F32R = mybir.dt.float32r
BF16 = mybir.dt.bfloat16
AF = mybir.ActivationFunctionType

N_TOTAL = 16384
C = 256        # input feature channels
OUT = 256      # output channels
NCORES = 8
NSH = N_TOTAL // NCORES   # 2048 rows per core
P = 128
KC = C // P    # contraction chunks
OC = OUT // P  # output-channel chunks
NTILE = 1024   # n-columns per pipeline tile
MMN = 512      # moving free dim per matmul (PSUM bank = 512 fp32)
NJ = NTILE // MMN
NT = NSH // NTILE
BN_EPS = 1e-5

# matmul operand mode: "f32" (4 cyc/row, exact), "f32r" (1 cyc/row,
# ~1.5e-4 rel err measured on HW), "bf16" (1 cyc/row, ~2.4e-3 rel err)
MODE = "f32r"

_cache = {}


def _build(mode):
    mdt = {"f32": F32, "f32r": F32R, "bf16": BF16}[mode]
    # Inputs consumed by matmuls are declared with the matmul dtype so the
    # load is a plain (fast, HWDGE) DMA and the operand-producer dtype
    # satisfies the FP32r rounding rule. For f32r the bits are plain fp32
    # (np dtype float32 either way).
    xdt = mdt if mode != "bf16" else F32
    nc = bacc.Bacc(None, target_bir_lowering=False, debug=True)

    x1_d = nc.declare_dram_parameter("x1t", [C, NSH], xdt, isOutput=False)
    x2_d = nc.declare_dram_parameter("x2t", [C, NSH], xdt, isOutput=False)
    wk_d = nc.declare_dram_parameter("wkT", [C, OUT], xdt, isOutput=False)
    wv1_d = nc.declare_dram_parameter("wv1T", [C, OUT], xdt, isOutput=False)
    wv2_d = nc.declare_dram_parameter("wv2T", [OUT, OUT], xdt, isOutput=False)
    wvc_d = nc.declare_dram_parameter("wvcT", [OUT, OUT], xdt, isOutput=False)
    sb_d = nc.declare_dram_parameter("sb", [OUT, 4], F32, isOutput=False)
    out_d = nc.declare_dram_parameter("outT", [OUT, NSH], F32, isOutput=True)

    with tile.TileContext(nc) as tc:
        with (
            tc.tile_pool(name="wpool", bufs=1) as wpool,
            tc.tile_pool(name="inpool", bufs=1) as inpool,
            tc.tile_pool(name="midpool", bufs=2) as midpool,
            tc.tile_pool(name="outpool", bufs=2) as outpool,
            tc.tile_pool(name="psum", bufs=1, space="PSUM") as psum,
        ):
            # -- PE warmup burst: keep the PE busy from the preamble's end
            # so the HAM clock gate opens sooner once real matmuls start.
            scratch = wpool.tile([P, MMN], BF16, tag="scratch")
            nc.gpsimd.memset(scratch, 0.0)
            wacc = psum.tile([P, MMN], F32, tag="acc", bufs=4)
            for _ in range(8):
                nc.tensor.matmul(wacc, scratch[:, :P], scratch,
                                 start=True, stop=True)
            # dummy sigmoid: hoist the (relu+sigmoid) ACT table load into
            # the DMA ramp instead of the first real activation
            dumm = wpool.tile([P, 1], F32, tag="dumm")
            nc.scalar.activation(dumm, scratch.bitcast(F32)[:, 0:1],
                                 AF.Sigmoid)

            def load_x(d, it, name):
                nsl = slice(it * NTILE, (it + 1) * NTILE)
                t = inpool.tile([P, KC, NTILE], mdt, tag=name)
                eng = nc.gpsimd if mode == "bf16" else nc.sync
                bi = eng.dma_start(
                    out=t,
                    in_=d.ap()[:, nsl].rearrange("(kc p) n -> p kc n", p=P))
                return t, bi

            x1, _ = load_x(x1_d, 0, "x1")
            x2, _ = load_x(x2_d, 0, "x2")

            def load_w(d, name):
                t = wpool.tile([P, KC, OUT], mdt, tag=name)
                nc.gpsimd.dma_start(
                    out=t, in_=d.ap().rearrange("(kc p) o -> p kc o", p=P))
                return t

            # sbt is tiny and feeds the h/val bias drains early - load first
            sbt = wpool.tile([P, OC, 4], F32, tag="sbt")
            nc.gpsimd.dma_start(
                out=sbt, in_=sb_d.ap().rearrange("(oc p) c -> p oc c", p=P))
            wk = load_w(wk_d, "wk")
            wv1 = load_w(wv1_d, "wv1")
            wvc = load_w(wvc_d, "wvc")
            wv2 = load_w(wv2_d, "wv2")

            def mm_group(w, rhs_tile, oc):
                # one PSUM region [P, NTILE] (NJ banks), KC*NJ matmuls
                acc = psum.tile([P, NTILE], F32, tag="acc", bufs=4)
                size = NTILE
                first = None
                for kc in range(KC):
                    for j in range(size // MMN):
                        bi = nc.tensor.matmul(
                            acc[:, j * MMN:(j + 1) * MMN],
                            w[:, kc, oc * P:(oc + 1) * P],
                            rhs_tile[:, kc, j * MMN:(j + 1) * MMN],
                            start=(kc == 0), stop=(kc == KC - 1))
                        first = first or bi
                return acc, first

            def as_f32(ap):
                return ap.bitcast(F32) if mode == "f32r" else ap

            for it in range(NT):
                nsl = slice(it * NTILE, (it + 1) * NTILE)
                if it > 0:
                    x1, _ = load_x(x1_d, it, "x1")
                    x2, _ = load_x(x2_d, it, "x2")

                pos = midpool.tile([P, OC, NTILE], mdt, tag="pos")
                h = midpool.tile([P, OC, NTILE], mdt, tag="h")
                vc = midpool.tile([P, OC, NTILE], F32, tag="vc")
                val = midpool.tile([P, OC, NTILE], F32, tag="val")
                prod = outpool.tile([P, OC, NTILE], F32, tag="prod")
                outt = outpool.tile([P, OC, NTILE], F32, tag="outt")

                # pos = relu(Wk @ x1)           (DVE: max(acc, 0))
                for oc in range(OC):
                    acc, _ = mm_group(wk, x1, oc)
                    nc.vector.tensor_scalar_max(pos[:, oc, :], acc, 0.0)
                # h = relu((s1*Wv1) @ x2 + b1)  (s1 folded on host; drains
                # split between ACT and DVE to balance engine load)
                for oc in range(OC):
                    acc, _ = mm_group(wv1, x2, oc)
                    if oc == 0:
                        nc.scalar.activation(h[:, oc, :], acc, AF.Relu,
                                             bias=sbt[:, oc, 0:1])
                    else:
                        nc.vector.tensor_scalar(h[:, oc, :], acc,
                                                sbt[:, oc, 0:1], 0.0,
                                                mybir.AluOpType.add,
                                                mybir.AluOpType.max)
                # vc = sigmoid(Wvc @ pos)       (ACT)
                for oc in range(OC):
                    acc, _ = mm_group(wvc, pos, oc)
                    nc.scalar.activation(vc[:, oc, :], acc, AF.Sigmoid)
                # val = relu((s2*Wv2) @ h + b2) (s2 folded on host; ACT)
                for oc in range(OC):
                    acc, _ = mm_group(wv2, h, oc)
                    nc.scalar.activation(val[:, oc, :], acc, AF.Relu,
                                         bias=sbt[:, oc, 2:3])
                # out = val + pos * vc  (mul DVE; adds split GpSimd/DVE on
                # the last tile so the tail runs in parallel)
                for oc in range(OC):
                    nc.vector.tensor_mul(prod[:, oc, :],
                                         as_f32(pos[:, oc, :]), vc[:, oc, :])
                last = it == NT - 1
                for oc in range(OC):
                    eng = nc.vector if (last and oc == 1) else nc.gpsimd
                    eng.tensor_add(outt[:, oc, :], prod[:, oc, :],
                                   val[:, oc, :])
                    nc.sync.dma_start(out=out_d.ap()[oc * P:(oc + 1) * P, nsl],
                                      in_=outt[:, oc, :])
    nc.finalize()
    return nc


def _prep(inputs):
    f = np.ascontiguousarray(np.asarray(inputs["features"], np.float32).T)
    f2 = np.ascontiguousarray(np.asarray(inputs["features2"], np.float32).T)
    wkT = np.ascontiguousarray(np.asarray(inputs["Wk"], np.float32).T)
    wvcT = np.ascontiguousarray(np.asarray(inputs["Wv_coef"], np.float32).T)

    eps = np.float32(BN_EPS)
    s1 = np.asarray(inputs["bn1_g"], np.float32) / np.sqrt(
        np.asarray(inputs["bn1_v"], np.float32) + eps)
    b1 = np.asarray(inputs["bn1_b"], np.float32) - np.asarray(
        inputs["bn1_m"], np.float32) * s1
    s2 = np.asarray(inputs["bn2_g"], np.float32) / np.sqrt(
        np.asarray(inputs["bn2_v"], np.float32) + eps)
    b2 = np.asarray(inputs["bn2_b"], np.float32) - np.asarray(
        inputs["bn2_m"], np.float32) * s2
    # bn scales fold into Wv1/Wv2 (per-output-channel row scales); biases
    # are applied on-chip.  sb columns: [b1, unused, b2, unused]
    wv1T = np.ascontiguousarray(
        (np.asarray(inputs["Wv1"], np.float32) * s1[:, None]).T)
    wv2T = np.ascontiguousarray(
        (np.asarray(inputs["Wv2"], np.float32) * s2[:, None]).T)
    sb = np.ascontiguousarray(
        np.stack([b1, s2, b2, np.zeros_like(b1)], axis=1).astype(np.float32))

    in_maps = []
    for i in range(NCORES):
        nsl = slice(i * NSH, (i + 1) * NSH)
        in_maps.append({
            "x1t": np.ascontiguousarray(f[:, nsl]),
            "x2t": np.ascontiguousarray(f2[:, nsl]),
            "wkT": wkT, "wv1T": wv1T, "wv2T": wv2T, "wvcT": wvcT,
            "sb": sb,
        })
    return in_maps


def _run(inputs, trace=False, trace_cores=None, tmpdir=None):
    if MODE not in _cache:
        _cache[MODE] = _build(MODE)
    nc = _cache[MODE]
    in_maps = _prep(inputs)
    kw = {}
    if trace:
        kw = dict(trace=True, trace_cores=trace_cores or [0], tmpdir=tmpdir)
    res = run_bass_kernel_spmd(nc, in_maps, core_ids=list(range(NCORES)), **kw)
    out = np.empty((N_TOTAL, OUT), np.float32)
    for i in range(NCORES):
        out[i * NSH:(i + 1) * NSH, :] = res.results[i]["outT"].T
    return out, res


def kernel(**inputs):
    out, _ = _run(inputs, trace=False)
    return out
